# revision 36
# baseline (speedup 1.0000x reference)
"""Trainium2 Bass kernel for the CustomAutoencoder problem.

Network (per batch row):
    h  = relu(einsum('gk,k->g', gw*mask, x[idx]) + gb)   # grouped gather-dense
    h  = relu(h @ w1 + b1); z = relu(h @ w2 + b2)
    d  = relu(z @ dw1 + db1); d = relu(d @ dw2 + db2)
    out = sigmoid(d @ dw3 + db3)

The gather+grouped-dense encoder is mathematically x @ A with
A[s, g] = sum_k (gw*mask)[g, k] * (idx[g, k] == s), so the whole model is a
dense MLP chain.  A is built on the host from the small param tensors
(replicated per the data-parallel sharding) and the batch dim is sharded
across 8 NeuronCores.

On-chip layout: activations are kept transposed [feature, batch]; x is
transposed on-device with PE transposes (f32r stream, 1.5 cyc/row).  All
matmul operands are bf16 (weights quantized on host, activation tiles
written bf16 by DVE/ACT) with fp32 PSUM accumulation; the rel-err budget
(2e-2) dwarfs the ~1e-3 this costs.  The last layer uses the transposed
activation as the stationary operand, yielding natural-layout output rows
directly; db3 enters as a K=1 ones-outer-product matmul.

DMA: input loads ride the Scalar (Activation) HWDGE queues in priority
order (x block 0 -> boot pack -> A pack -> rest pack -> row pack -> x
block 1); output stores ride the Sync queues one [128, 512] row-block at a
time so the tail overlaps the final sigmoids.  Each engine stays within
its 8 HWDGE queues so every DMA needs at most one sync wait (walrus
single-wait rule).  The tiny "touch" matmuls / scalar copies pre-advance
each engine's observed vector clock past DMA producers for the same
reason.
"""

import os
import sys

sys.path.insert(0, "/opt/trn_rl_repo")

import numpy as np
import ml_dtypes

import concourse.bass as bass
import concourse.tile as tile
from concourse import mybir
from concourse.bass import ts
from concourse.bass_utils import run_bass_kernel_spmd
from concourse.tile_rust import add_dep_helper

F32 = mybir.dt.float32
F32R = mybir.dt.float32r
BF16 = mybir.dt.bfloat16
AF = mybir.ActivationFunctionType
BF16_NP = ml_dtypes.bfloat16

B = 8192          # full batch
S = 512           # sample size (input/output features)
G = 510           # number of groups
GP = 512          # G padded to a multiple of 128
HID = 128
LAT = 32
HID2 = 256
NCORES = 8
BC = B // NCORES  # rows per core
BBLK = 512        # batch columns per block (PSUM free-dim max for fp32)
NBLK = BC // BBLK

# boot pack [128, BOOT_COLS] fp32: ident + per-partition biases.
IDENT_BASE = 0        # 128 x 128 fp32 identity (bitcast f32r at use)
BIAS_BASE = 128       # cols +0..3 gb chunks, +4 b1, +5 b2 (rows<32),
                      # +6 db1, +7..8 db2 chunks
BOOT_COLS = 137
# A pack [128, 2048] f32r: 4 x 512 (A[fi*128+p, g]).  The walrus verifier
# rejects mixed 32/16-bit matmul operands and bf16 moving operands stream
# SLOWER than f32r on this part (258 vs 213 ns per 512 rows), so everything
# stays f32r.
A_COLS = 2048
# rest pack [128, REST_COLS] f32r (row 0 tail: ones row + db3, so the
# total DMA count stays at 8 = the HWDGE queue budget):
W1_BASE = 0           # 4 x 128  (w1[fi*128+p, m])
W2_BASE = 512         # 128 x 32
DW1_BASE = 544        # 32 x 128 (rows 0-31)
DW2_BASE = 672        # 128 x 256
DW3_BASE = 928        # 2 x 512  (dw3[kj*128+p, s])
ONES_BASE = 1952      # row 0, 128 cols of 1.0
DB3_BASE = 2080       # row 0, 512 cols
REST_COLS = 2592

_CACHE: dict = {}
last_results = None


def _touch(nc, scratch, tl):
    """1x1 PE matmul reading a corner of `tl`: advances the PE engine's
    observed vector clock past tl's producer (walrus S3_LW single-wait)."""
    return nc.tensor.matmul(
        scratch[0:1, 0:2], tl[0:1, 0:1], tl[0:1, 0:2], start=True, stop=True
    )


_STOUCH_IDX = [0]


def _stouch(nc, sdump, tl):
    """Scalar-engine equivalent of _touch for ACT-consumed (bias) tiles."""
    k = _STOUCH_IDX[0] % 32
    _STOUCH_IDX[0] += 1
    return nc.scalar.copy(out=sdump[0:1, k : k + 1], in_=tl[0:1, 0:1])


_VTOUCH_IDX = [0]


def _vtouch(nc, vdump, tl):
    """Vector-engine equivalent of _touch for DVE-consumed tiles."""
    k = _VTOUCH_IDX[0] % 32
    _VTOUCH_IDX[0] += 1
    return nc.vector.tensor_copy(vdump[0:1, k : k + 1], tl[0:1, 0:1])


class SplitDrainTileContext(tile.TileContext):
    """TileContext whose kernel-tail drain carries at most one sync wait per
    instruction: this walrus build rejects >1 sync wait on any instruction,
    and the stock tail drain aggregates one wait per active proc."""

    def _drain_and_barrier(self, tick_clock, wait_clock):
        from concourse.vector_clock import ScopedClock, VectorClock

        gc = tick_clock.global_clock
        n = len(gc)
        for p in range(n):
            t = gc[p]
            if t == 0:
                continue
            single = [0] * n
            single[p] = t
            nop = self.nc.sync.nop(nofuse=True, hint="split_drain_wait")
            wait_clock.add_sem_waits(
                nop.ins, ScopedClock({None: VectorClock(single)})
            )
        # The per-proc nops above already enforce every outstanding tick in
        # SP program order, so the drain itself needs no waits.
        self.nc.sync.drain()
        self.nc.all_engine_barrier()
        assert self.sems is not None
        popped = self.nc._tile_sem_poison_stack.pop()
        assert popped is self._sem_poison
        self.nc.clear_and_free_semaphores(list(self.sems.allocated().values()))
        self.nc.all_engine_barrier()


def _build_program():
    nc = bass.Bass()

    # x and bootpack are declared f32r: the DMA then satisfies the BIR
    # verifier's "f32r consumers need f32r producers" rule, and the PE
    # transposes stream them at 1.5 cyc/row instead of fp32's 2.
    x_d = nc.declare_dram_parameter("x", [BC, S], F32R, isOutput=False)
    boot_d = nc.declare_dram_parameter("bootpack", [128, BOOT_COLS], F32R,
                                       isOutput=False)
    a_d = nc.declare_dram_parameter("apack", [128, A_COLS], F32R,
                                    isOutput=False)
    rest_d = nc.declare_dram_parameter("restpack", [128, REST_COLS], F32R,
                                       isOutput=False)
    out_d = nc.declare_dram_parameter("out", [BC, S], F32, isOutput=True)

    x_v = x_d.rearrange("(k i p) s -> k p i s", p=128, i=4)    # [NBLK,128,4,512]
    out_v = out_d.rearrange("(k i p) s -> k p i s", p=128, i=4)

    with SplitDrainTileContext(nc) as tc:
        with (
            tc.tile_pool(name="weights", bufs=1) as wp,
            tc.tile_pool(name="xin", bufs=2) as xp,
            tc.tile_pool(name="xt", bufs=8) as xtp,
            tc.tile_pool(name="acts", bufs=4) as ap_,
            tc.tile_pool(name="h1", bufs=8) as h1p,
            tc.tile_pool(name="outs", bufs=2) as op_,
            tc.tile_pool(name="pt", bufs=2, space="PSUM") as ptp,
            tc.tile_pool(name="p1", bufs=3, space="PSUM") as p1p,
            tc.tile_pool(name="pmid", bufs=2, space="PSUM") as pmp,
            tc.tile_pool(name="psc", bufs=1, space="PSUM") as pscp,
        ):
            # l6 ping-pongs on the pt pool (free once the transposes are
            # done), so p1 gets a third bank to decouple the L1 matmuls from
            # the DVE h1 read-out.
            scratch = pscp.tile([1, 2], F32)
            sdump = wp.tile([1, 32], F32, tag="sdump")
            vdump = wp.tile([1, 32], F32, tag="vdump")
            _STOUCH_IDX[0] = 0
            _VTOUCH_IDX[0] = 0

            # Load priority: x block 0 gates the first transposes, then the
            # small boot pack (ident + biases), then A (gates L1).
            xbs = []
            xb = xp.tile([128, 4, BBLK], F32R, tag="xb")
            nc.scalar.dma_start(out=xb[:], in_=x_v[0])
            xbs.append(xb)
            boot_sb = wp.tile([128, BOOT_COLS], F32R, tag="bootpack")
            nc.scalar.dma_start(out=boot_sb[:], in_=boot_d[:, :])
            _touch(nc, scratch, boot_sb)
            _stouch(nc, sdump, boot_sb[:, 0:1].bitcast(F32))
            _vtouch(nc, vdump, boot_sb[:, 0:1].bitcast(F32))
            a_sb = wp.tile([128, A_COLS], F32R, tag="apack")
            nc.scalar.dma_start(out=a_sb[:], in_=a_d[:, :])
            rest_sb = wp.tile([128, REST_COLS], F32R, tag="restpack")
            nc.scalar.dma_start(out=rest_sb[:], in_=rest_d[:, :])
            xb = xp.tile([128, 4, BBLK], F32R, tag="xb")
            nc.scalar.dma_start(out=xb[:], in_=x_v[1])
            xbs.append(xb)

            ident = boot_sb[:, IDENT_BASE : IDENT_BASE + 128]
            ones = rest_sb[0:1, ONES_BASE : ONES_BASE + 128]
            db3_sl = rest_sb[0:1, DB3_BASE : DB3_BASE + S]   # rhs [1, 512]

            def a_sl(fi, gj):        # lhsT [128, 128]
                c = fi * 512 + gj * 128
                return a_sb[:, c : c + 128]

            def w1_sl(fi):           # lhsT [128, 128]
                return rest_sb[:, W1_BASE + fi * 128 : W1_BASE + (fi + 1) * 128]

            w2_sl = rest_sb[:, W2_BASE : W2_BASE + LAT]          # [128, 32]
            dw1_sl = rest_sb[0:LAT, DW1_BASE : DW1_BASE + HID]   # [32, 128]

            def dw2_sl(j):           # lhsT [128, 128]
                return rest_sb[:, DW2_BASE + j * 128 : DW2_BASE + (j + 1) * 128]

            def dw3_sl(kj):          # rhs [128, 512]
                return rest_sb[:, DW3_BASE + kj * 512 : DW3_BASE + (kj + 1) * 512]

            def bias_col(i, rows=128):
                return boot_sb[0:rows, BIAS_BASE + i : BIAS_BASE + i + 1].bitcast(F32)

            gb_b = [bias_col(i) for i in range(4)]
            b1_b = bias_col(4)
            b2_b = bias_col(5, rows=LAT)
            db1_b = bias_col(6)
            db2_b = [bias_col(7 + j) for j in range(2)]

            st_ = {"xt": {}, "h1": {}, "h2": {}, "z": {}, "d1": {}, "d2": {},
                   "ob": {}}

            def transposes(blk):
                xbr = xbs[blk]
                xtch = _touch(nc, scratch, xbr[:, 0, :])
                xt_sb = []
                for fj in range(4):
                    pt = ptp.tile([128, BBLK], F32R)
                    for bi in range(4):
                        tp = nc.tensor.transpose(
                            pt[:, ts(bi, 128)], xbr[:, bi, ts(fj, 128)], ident
                        )
                        if bi == 0:
                            add_dep_helper(tp.ins, xtch.ins, sync=False,
                                           reason="transpose after x touch")
                    st = xtp.tile([128, BBLK], F32R)
                    nc.vector.tensor_copy(st[:], pt[:])
                    # PE observes the DVE tick so the next transpose group
                    # reusing this PSUM slot needs at most one sync wait.
                    _touch(nc, scratch, st)
                    xt_sb.append(st)
                st_["xt"][blk] = xt_sb

            def l1(blk, gj):
                # h1T[gj] = relu(sum_fi A[fi, gj].T @ xT[fi] + gb[gj])
                ps = p1p.tile([128, BBLK], F32, tag="p1")
                for fi in range(4):
                    nc.tensor.matmul(
                        ps[:], a_sl(fi, gj), st_["xt"][blk][fi][:],
                        start=(fi == 0), stop=(fi == 3),
                    )
                h = h1p.tile([128, BBLK], F32R)
                nc.vector.tensor_scalar(
                    h[:], ps[:], gb_b[gj], 0.0,
                    op0=mybir.AluOpType.add, op1=mybir.AluOpType.max,
                )
                st_["h1"].setdefault(blk, []).append(h)
                if gj == 3:
                    # PE observes the DVE ticks of the h1 producers, so L2/L6
                    # matmuls keep at most one sync wait.
                    _touch(nc, scratch, h)

            def l2(blk):
                ps = pmp.tile([128, BBLK], F32, tag="pmid")
                for fi in range(4):
                    nc.tensor.matmul(
                        ps[:], w1_sl(fi), st_["h1"][blk][fi][:],
                        start=(fi == 0), stop=(fi == 3),
                    )
                h2 = ap_.tile([HID, BBLK], F32R, tag="h2")
                nc.scalar.activation(h2[:], ps[:], AF.Relu, bias=b1_b)
                st_["h2"][blk] = h2

            def l3(blk):
                ps = pmp.tile([LAT, BBLK], F32, tag="pmid")
                nc.tensor.matmul(ps[:], w2_sl, st_["h2"][blk][:], start=True,
                                 stop=True)
                z = ap_.tile([LAT, BBLK], F32R, tag="z")
                nc.scalar.activation(z[:], ps[:], AF.Relu, bias=b2_b)
                st_["z"][blk] = z

            def l4(blk):
                ps = pmp.tile([HID, BBLK], F32, tag="pmid")
                nc.tensor.matmul(ps[:], dw1_sl, st_["z"][blk][:], start=True,
                                 stop=True)
                d1 = ap_.tile([HID, BBLK], F32R, tag="d1")
                nc.scalar.activation(d1[:], ps[:], AF.Relu, bias=db1_b)
                st_["d1"][blk] = d1

            def l5(blk, j):
                ps = pmp.tile([128, BBLK], F32, tag="pmid")
                nc.tensor.matmul(ps[:], dw2_sl(j), st_["d1"][blk][:],
                                 start=True, stop=True)
                d2 = ap_.tile([128, BBLK], F32R, tag=f"d2_{j}")
                nc.scalar.activation(d2[:], ps[:], AF.Relu, bias=db2_b[j])
                st_["d2"].setdefault(blk, []).append(d2)

            def l6(blk, bi):
                # out[bi] = sigmoid(sum_j d2T[j][:, bi].T @ dw3[j] + db3),
                # natural layout directly; db3 enters as a K=1 ones-outer-
                # product matmul inside the accumulation group.
                if bi == 0:
                    ob = op_.tile([128, 4, S], F32, tag="ob")
                    st_["ob"][blk] = ob
                d2_sb = st_["d2"][blk]
                ps = ptp.tile([128, S], F32, tag="pt")
                nc.tensor.matmul(
                    ps[:], d2_sb[0][:, ts(bi, 128)], dw3_sl(0),
                    start=True, stop=False,
                )
                nc.tensor.matmul(ps[:], ones, db3_sl, start=False, stop=False)
                nc.tensor.matmul(
                    ps[:], d2_sb[1][:, ts(bi, 128)], dw3_sl(1),
                    start=False, stop=True,
                )
                nc.scalar.activation(st_["ob"][blk][:, bi, :], ps[:],
                                     AF.Sigmoid)

            # Software-pipelined emission: per-engine streams execute in
            # program order, so blk1's transposes/L1 are interleaved into
            # blk0's mid-layer chain to keep the PE busy while ACT/DVE work.
            transposes(0)
            _touch(nc, scratch, a_sb)
            for gj in range(4):
                l1(0, gj)
            _touch(nc, scratch, rest_sb)
            l2(0)
            l3(0)
            l4(0)
            transposes(1)
            l5(0, 0)
            l5(0, 1)
            for gj in range(4):
                l1(1, gj)
            l2(1)
            l3(1)
            l4(1)
            def store(blk, halves=False):
                # 5 loads + 3 stores fit the 8 HWDGE queues, so no DMA needs
                # a queue-reuse wait on top of its data wait (walrus allows
                # only one sync wait per DMA).  The final store is split so
                # the first half streams out while the last sigmoids run.
                ob = st_["ob"][blk]
                if halves:
                    nc.sync.dma_start(out=out_v[blk, :, 0:2], in_=ob[:, 0:2])
                    nc.sync.dma_start(out=out_v[blk, :, 2:4], in_=ob[:, 2:4])
                else:
                    nc.sync.dma_start(out=out_v[blk], in_=ob[:])

            l5(1, 0)
            l5(1, 1)
            for bi in range(4):
                l6(0, bi)
            store(0)
            for bi in range(4):
                l6(1, bi)
            store(1, halves=True)

    return nc


def _get_program():
    if "prog" not in _CACHE:
        _CACHE["prog"] = _build_program()
    return _CACHE["prog"]


def _pack_params(inputs):
    gw = np.asarray(inputs["gw"], dtype=np.float32)
    gb = np.asarray(inputs["gb"], dtype=np.float32)
    idx = np.asarray(inputs["idx"], dtype=np.int64)
    mask = np.asarray(inputs["mask"], dtype=np.float32)
    w1 = np.asarray(inputs["w1"], dtype=np.float32)
    b1 = np.asarray(inputs["b1"], dtype=np.float32)
    w2 = np.asarray(inputs["w2"], dtype=np.float32)
    b2 = np.asarray(inputs["b2"], dtype=np.float32)
    dw1 = np.asarray(inputs["dw1"], dtype=np.float32)
    db1 = np.asarray(inputs["db1"], dtype=np.float32)
    dw2 = np.asarray(inputs["dw2"], dtype=np.float32)
    db2 = np.asarray(inputs["db2"], dtype=np.float32)
    dw3 = np.asarray(inputs["dw3"], dtype=np.float32)
    db3 = np.asarray(inputs["db3"], dtype=np.float32)

    g, k = idx.shape
    assert g == G

    # Fold gather + grouped Dense(1) into a dense [S, GP] matrix.
    a_mat = np.zeros((S, GP), dtype=np.float32)
    gwm = (gw * mask).astype(np.float32)
    cols = np.repeat(np.arange(g, dtype=np.int64), k)
    np.add.at(a_mat, (idx.reshape(-1), cols), gwm.reshape(-1))

    bootpack = np.zeros((128, BOOT_COLS), dtype=np.float32)
    bootpack[:, IDENT_BASE : IDENT_BASE + 128] = np.eye(128, dtype=np.float32)
    gb_pad = np.zeros(GP, np.float32)
    gb_pad[:g] = gb
    for i in range(4):
        bootpack[:, BIAS_BASE + i] = gb_pad[i * 128 : (i + 1) * 128]
    bootpack[:, BIAS_BASE + 4] = b1
    bootpack[:LAT, BIAS_BASE + 5] = b2
    bootpack[:, BIAS_BASE + 6] = db1
    bootpack[:, BIAS_BASE + 7] = db2[:128]
    bootpack[:, BIAS_BASE + 8] = db2[128:]

    apack = np.zeros((128, A_COLS), dtype=np.float32)
    for fi in range(4):
        apack[:, fi * 512 : (fi + 1) * 512] = a_mat[fi * 128 : (fi + 1) * 128, :]

    restpack = np.zeros((128, REST_COLS), dtype=np.float32)
    w1_pad = np.zeros((GP, HID), dtype=np.float32)
    w1_pad[:g] = w1
    for fi in range(4):
        restpack[:, W1_BASE + fi * 128 : W1_BASE + (fi + 1) * 128] = \
            w1_pad[fi * 128 : (fi + 1) * 128, :]
    restpack[:, W2_BASE : W2_BASE + LAT] = w2
    restpack[:LAT, DW1_BASE : DW1_BASE + HID] = dw1
    restpack[:, DW2_BASE : DW2_BASE + HID2] = dw2
    for kj in range(2):
        restpack[:, DW3_BASE + kj * 512 : DW3_BASE + (kj + 1) * 512] = \
            dw3[kj * 128 : (kj + 1) * 128, :]
    restpack[0, ONES_BASE : ONES_BASE + 128] = 1.0
    restpack[0, DB3_BASE : DB3_BASE + S] = db3

    return bootpack, apack, restpack


def kernel(**inputs) -> np.ndarray:
    global last_results

    x = np.ascontiguousarray(np.asarray(inputs["x"], dtype=np.float32))
    assert x.shape == (B, S), x.shape
    bootpack, apack, restpack = _pack_params(inputs)

    in_maps = [
        {"x": x[c * BC : (c + 1) * BC], "bootpack": bootpack,
         "apack": apack, "restpack": restpack}
        for c in range(NCORES)
    ]

    nc = _get_program()
    trace = os.environ.get("KERNEL_TRACE", "0") == "1"
    res = run_bass_kernel_spmd(nc, in_maps, list(range(NCORES)), trace=trace)
    last_results = res
    out = np.concatenate([r["out"] for r in res.results], axis=0)
    return out.astype(np.float32)


if __name__ == "__main__":
    rng = np.random.RandomState(0)
    demo = {
        "x": rng.rand(B, S).astype(np.float32),
        "gw": rng.randn(G, 30).astype(np.float32),
        "gb": rng.randn(G).astype(np.float32) * 0.1,
        "idx": rng.randint(0, S, (G, 30)).astype(np.int32),
        "mask": (rng.rand(G, 30) > 0.5).astype(np.float32),
        "w1": rng.randn(G, HID).astype(np.float32) * 0.04,
        "b1": rng.randn(HID).astype(np.float32) * 0.1,
        "w2": rng.randn(HID, LAT).astype(np.float32) * 0.09,
        "b2": rng.randn(LAT).astype(np.float32) * 0.1,
        "dw1": rng.randn(LAT, HID).astype(np.float32) * 0.18,
        "db1": rng.randn(HID).astype(np.float32) * 0.1,
        "dw2": rng.randn(HID, HID2).astype(np.float32) * 0.09,
        "db2": rng.randn(HID2).astype(np.float32) * 0.1,
        "dw3": rng.randn(HID2, S).astype(np.float32) * 0.06,
        "db3": rng.randn(S).astype(np.float32) * 0.1,
    }
    out = kernel(**demo)
    print("out", out.shape, out.dtype, float(out.mean()))


# revision 40
# speedup vs baseline: 1.0751x; 1.0751x over previous
"""Trainium2 Bass kernel for the CustomAutoencoder problem.

Network (per batch row):
    h  = relu(einsum('gk,k->g', gw*mask, x[idx]) + gb)   # grouped gather-dense
    h  = relu(h @ w1 + b1); z = relu(h @ w2 + b2)
    d  = relu(z @ dw1 + db1); d = relu(d @ dw2 + db2)
    out = sigmoid(d @ dw3 + db3)

The gather+grouped-dense encoder is mathematically x @ A with
A[s, g] = sum_k (gw*mask)[g, k] * (idx[g, k] == s), so the whole model is a
dense MLP chain.  A is built on the host from the small param tensors
(replicated per the data-parallel sharding) and the batch dim is sharded
across 8 NeuronCores.

On-chip layout: activations are kept transposed [feature, batch]; x is
transposed on-device with PE transposes (f32r stream, 1.5 cyc/row).  All
matmul operands are bf16 (weights quantized on host, activation tiles
written bf16 by DVE/ACT) with fp32 PSUM accumulation; the rel-err budget
(2e-2) dwarfs the ~1e-3 this costs.  The last layer uses the transposed
activation as the stationary operand, yielding natural-layout output rows
directly; db3 enters as a K=1 ones-outer-product matmul.

DMA: input loads ride the Scalar (Activation) HWDGE queues in priority
order (x block 0 -> boot pack -> A pack -> rest pack -> row pack -> x
block 1); output stores ride the Sync queues one [128, 512] row-block at a
time so the tail overlaps the final sigmoids.  Each engine stays within
its 8 HWDGE queues so every DMA needs at most one sync wait (walrus
single-wait rule).  The tiny "touch" matmuls / scalar copies pre-advance
each engine's observed vector clock past DMA producers for the same
reason.
"""

import os
import sys

sys.path.insert(0, "/opt/trn_rl_repo")

import numpy as np
import ml_dtypes

import concourse.bass as bass
import concourse.tile as tile
from concourse import mybir
from concourse.bass import ts
from concourse.bass_utils import run_bass_kernel_spmd
from concourse.tile_rust import add_dep_helper

F32 = mybir.dt.float32
F32R = mybir.dt.float32r
BF16 = mybir.dt.bfloat16
AF = mybir.ActivationFunctionType
BF16_NP = ml_dtypes.bfloat16

B = 8192          # full batch
S = 512           # sample size (input/output features)
G = 510           # number of groups
GP = 512          # G padded to a multiple of 128
HID = 128
LAT = 32
HID2 = 256
NCORES = 8
BC = B // NCORES  # rows per core
BBLK = 512        # batch columns per block (PSUM free-dim max for fp32)
NBLK = BC // BBLK

# boot pack [128, BOOT_COLS] fp32: ident + per-partition biases.
IDENT_BASE = 0        # 128 x 128 fp32 identity (bitcast f32r at use)
BIAS_BASE = 128       # cols +0..3 gb chunks, +4 b1, +5 b2 (rows<32),
                      # +6 db1, +7..8 db2 chunks
BOOT_COLS = 137
# A pack [128, 2048] f32r: 4 x 512 (A[fi*128+p, g]).  The walrus verifier
# rejects mixed 32/16-bit matmul operands and bf16 moving operands stream
# SLOWER than f32r on this part (258 vs 213 ns per 512 rows), so everything
# stays f32r.
A_COLS = 2048
# rest pack [128, REST_COLS] f32r (row 0 tail: ones row + db3, so the
# total DMA count stays at 8 = the HWDGE queue budget):
W1_BASE = 0           # 4 x 128  (w1[fi*128+p, m])
W2_BASE = 512         # 128 x 32
DW1_BASE = 544        # 32 x 128 (rows 0-31)
DW2_BASE = 672        # 128 x 256
DW3_BASE = 928        # 2 x 512  (dw3[kj*128+p, s])
ONES_BASE = 1952      # row 0, 128 cols of 1.0
DB3_BASE = 2080       # row 0, 512 cols
REST_COLS = 2592

_CACHE: dict = {}
last_results = None


def _touch(nc, scratch, tl):
    """1x1 PE matmul reading a corner of `tl`: advances the PE engine's
    observed vector clock past tl's producer (walrus S3_LW single-wait)."""
    return nc.tensor.matmul(
        scratch[0:1, 0:2], tl[0:1, 0:1], tl[0:1, 0:2], start=True, stop=True
    )


_STOUCH_IDX = [0]


def _stouch(nc, sdump, tl):
    """Scalar-engine equivalent of _touch for ACT-consumed (bias) tiles."""
    k = _STOUCH_IDX[0] % 32
    _STOUCH_IDX[0] += 1
    return nc.scalar.copy(out=sdump[0:1, k : k + 1], in_=tl[0:1, 0:1])


_VTOUCH_IDX = [0]


def _vtouch(nc, vdump, tl):
    """Vector-engine equivalent of _touch for DVE-consumed tiles."""
    k = _VTOUCH_IDX[0] % 32
    _VTOUCH_IDX[0] += 1
    return nc.vector.tensor_copy(vdump[0:1, k : k + 1], tl[0:1, 0:1])


class SplitDrainTileContext(tile.TileContext):
    """TileContext whose kernel-tail drain carries at most one sync wait per
    instruction: this walrus build rejects >1 sync wait on any instruction,
    and the stock tail drain aggregates one wait per active proc."""

    def _drain_and_barrier(self, tick_clock, wait_clock):
        from concourse.vector_clock import ScopedClock, VectorClock

        gc = tick_clock.global_clock
        n = len(gc)
        for p in range(n):
            t = gc[p]
            if t == 0:
                continue
            single = [0] * n
            single[p] = t
            nop = self.nc.sync.nop(nofuse=True, hint="split_drain_wait")
            wait_clock.add_sem_waits(
                nop.ins, ScopedClock({None: VectorClock(single)})
            )
        # The per-proc nops above already enforce every outstanding tick in
        # SP program order, so the drain itself needs no waits.
        self.nc.sync.drain()
        self.nc.all_engine_barrier()
        assert self.sems is not None
        popped = self.nc._tile_sem_poison_stack.pop()
        assert popped is self._sem_poison
        self.nc.clear_and_free_semaphores(list(self.sems.allocated().values()))
        self.nc.all_engine_barrier()


def _build_program():
    nc = bass.Bass()

    # x stays plain fp32: fp32-input PE transposes measure FASTER than the
    # f32r variant (LDWEIGHTS ~74 ns vs ~330 ns per 128x128 chunk) and keep
    # the PE stream dense through the warm-up window.
    x_d = nc.declare_dram_parameter("x", [BC, S], F32, isOutput=False)
    boot_d = nc.declare_dram_parameter("bootpack", [128, BOOT_COLS], F32R,
                                       isOutput=False)
    a_d = nc.declare_dram_parameter("apack", [128, A_COLS], F32R,
                                    isOutput=False)
    rest_d = nc.declare_dram_parameter("restpack", [128, REST_COLS], F32R,
                                       isOutput=False)
    out_d = nc.declare_dram_parameter("out", [BC, S], F32, isOutput=True)

    x_v = x_d.rearrange("(k i p) s -> k p i s", p=128, i=4)    # [NBLK,128,4,512]
    out_v = out_d.rearrange("(k i p) s -> k p i s", p=128, i=4)

    with SplitDrainTileContext(nc) as tc:
        with (
            tc.tile_pool(name="weights", bufs=1) as wp,
            tc.tile_pool(name="xin", bufs=2) as xp,
            tc.tile_pool(name="xt", bufs=8) as xtp,
            tc.tile_pool(name="acts", bufs=4) as ap_,
            tc.tile_pool(name="h1", bufs=8) as h1p,
            tc.tile_pool(name="outs", bufs=2) as op_,
            tc.tile_pool(name="pt", bufs=2, space="PSUM") as ptp,
            tc.tile_pool(name="p1", bufs=3, space="PSUM") as p1p,
            tc.tile_pool(name="pmid", bufs=2, space="PSUM") as pmp,
            tc.tile_pool(name="psc", bufs=1, space="PSUM") as pscp,
        ):
            # l6 ping-pongs on the pt pool (free once the transposes are
            # done), so p1 gets a third bank to decouple the L1 matmuls from
            # the DVE h1 read-out.
            scratch = pscp.tile([1, 2], F32)
            sdump = wp.tile([1, 32], F32, tag="sdump")
            vdump = wp.tile([1, 32], F32, tag="vdump")
            _STOUCH_IDX[0] = 0
            _VTOUCH_IDX[0] = 0

            # Load priority: x block 0 gates the first transposes, then the
            # small boot pack (ident + biases), then A (gates L1).
            xbs = []
            xb = xp.tile([128, 4, BBLK], F32, tag="xb")
            nc.scalar.dma_start(out=xb[:], in_=x_v[0])
            xbs.append(xb)
            boot_sb = wp.tile([128, BOOT_COLS], F32R, tag="bootpack")
            nc.scalar.dma_start(out=boot_sb[:], in_=boot_d[:, :])
            _touch(nc, scratch, boot_sb)
            _stouch(nc, sdump, boot_sb[:, 0:1].bitcast(F32))
            _vtouch(nc, vdump, boot_sb[:, 0:1].bitcast(F32))
            a_sb = wp.tile([128, A_COLS], F32R, tag="apack")
            nc.scalar.dma_start(out=a_sb[:], in_=a_d[:, :])
            rest_sb = wp.tile([128, REST_COLS], F32R, tag="restpack")
            nc.scalar.dma_start(out=rest_sb[:], in_=rest_d[:, :])
            xb = xp.tile([128, 4, BBLK], F32, tag="xb")
            nc.scalar.dma_start(out=xb[:], in_=x_v[1])
            xbs.append(xb)

            ident = boot_sb[:, IDENT_BASE : IDENT_BASE + 128].bitcast(F32)
            ones = rest_sb[0:1, ONES_BASE : ONES_BASE + 128]
            db3_sl = rest_sb[0:1, DB3_BASE : DB3_BASE + S]   # rhs [1, 512]

            def a_sl(fi, gj):        # lhsT [128, 128]
                c = fi * 512 + gj * 128
                return a_sb[:, c : c + 128]

            def w1_sl(fi):           # lhsT [128, 128]
                return rest_sb[:, W1_BASE + fi * 128 : W1_BASE + (fi + 1) * 128]

            w2_sl = rest_sb[:, W2_BASE : W2_BASE + LAT]          # [128, 32]
            dw1_sl = rest_sb[0:LAT, DW1_BASE : DW1_BASE + HID]   # [32, 128]

            def dw2_sl(j):           # lhsT [128, 128]
                return rest_sb[:, DW2_BASE + j * 128 : DW2_BASE + (j + 1) * 128]

            def dw3_sl(kj):          # rhs [128, 512]
                return rest_sb[:, DW3_BASE + kj * 512 : DW3_BASE + (kj + 1) * 512]

            def bias_col(i, rows=128):
                return boot_sb[0:rows, BIAS_BASE + i : BIAS_BASE + i + 1].bitcast(F32)

            gb_b = [bias_col(i) for i in range(4)]
            b1_b = bias_col(4)
            b2_b = bias_col(5, rows=LAT)
            db1_b = bias_col(6)
            db2_b = [bias_col(7 + j) for j in range(2)]

            st_ = {"xt": {}, "h1": {}, "h2": {}, "z": {}, "d1": {}, "d2": {},
                   "ob": {}}

            def transposes(blk):
                xbr = xbs[blk]
                xtch = _touch(nc, scratch, xbr[:, 0, :])
                xt_sb = []
                for fj in range(4):
                    pt = ptp.tile([128, BBLK], F32)
                    for bi in range(4):
                        tp = nc.tensor.transpose(
                            pt[:, ts(bi, 128)], xbr[:, bi, ts(fj, 128)], ident
                        )
                        if bi == 0:
                            add_dep_helper(tp.ins, xtch.ins, sync=False,
                                           reason="transpose after x touch")
                    st = xtp.tile([128, BBLK], F32R)
                    nc.vector.tensor_copy(st[:], pt[:])
                    # PE observes the DVE tick so the next transpose group
                    # reusing this PSUM slot needs at most one sync wait.
                    _touch(nc, scratch, st)
                    xt_sb.append(st)
                st_["xt"][blk] = xt_sb

            def l1(blk, gj):
                # h1T[gj] = relu(sum_fi A[fi, gj].T @ xT[fi] + gb[gj])
                ps = p1p.tile([128, BBLK], F32, tag="p1")
                for fi in range(4):
                    nc.tensor.matmul(
                        ps[:], a_sl(fi, gj), st_["xt"][blk][fi][:],
                        start=(fi == 0), stop=(fi == 3),
                    )
                h = h1p.tile([128, BBLK], F32R)
                nc.vector.tensor_scalar(
                    h[:], ps[:], gb_b[gj], 0.0,
                    op0=mybir.AluOpType.add, op1=mybir.AluOpType.max,
                )
                st_["h1"].setdefault(blk, []).append(h)
                if gj == 3:
                    # PE observes the DVE ticks of the h1 producers, so L2/L6
                    # matmuls keep at most one sync wait.
                    _touch(nc, scratch, h)

            def l2(blk):
                ps = pmp.tile([128, BBLK], F32, tag="pmid")
                for fi in range(4):
                    nc.tensor.matmul(
                        ps[:], w1_sl(fi), st_["h1"][blk][fi][:],
                        start=(fi == 0), stop=(fi == 3),
                    )
                h2 = ap_.tile([HID, BBLK], F32R, tag="h2")
                nc.scalar.activation(h2[:], ps[:], AF.Relu, bias=b1_b)
                st_["h2"][blk] = h2

            def l3(blk):
                ps = pmp.tile([LAT, BBLK], F32, tag="pmid")
                nc.tensor.matmul(ps[:], w2_sl, st_["h2"][blk][:], start=True,
                                 stop=True)
                z = ap_.tile([LAT, BBLK], F32R, tag="z")
                nc.scalar.activation(z[:], ps[:], AF.Relu, bias=b2_b)
                st_["z"][blk] = z

            def l4(blk):
                ps = pmp.tile([HID, BBLK], F32, tag="pmid")
                nc.tensor.matmul(ps[:], dw1_sl, st_["z"][blk][:], start=True,
                                 stop=True)
                d1 = ap_.tile([HID, BBLK], F32R, tag="d1")
                nc.scalar.activation(d1[:], ps[:], AF.Relu, bias=db1_b)
                st_["d1"][blk] = d1

            def l5(blk, j):
                ps = pmp.tile([128, BBLK], F32, tag="pmid")
                nc.tensor.matmul(ps[:], dw2_sl(j), st_["d1"][blk][:],
                                 start=True, stop=True)
                d2 = ap_.tile([128, BBLK], F32R, tag=f"d2_{j}")
                nc.scalar.activation(d2[:], ps[:], AF.Relu, bias=db2_b[j])
                st_["d2"].setdefault(blk, []).append(d2)

            def l6(blk, bi):
                # out[bi] = sigmoid(sum_j d2T[j][:, bi].T @ dw3[j] + db3),
                # natural layout directly; db3 enters as a K=1 ones-outer-
                # product matmul inside the accumulation group.
                if bi == 0:
                    ob = op_.tile([128, 4, S], F32, tag="ob")
                    st_["ob"][blk] = ob
                d2_sb = st_["d2"][blk]
                ps = ptp.tile([128, S], F32, tag="pt")
                nc.tensor.matmul(
                    ps[:], d2_sb[0][:, ts(bi, 128)], dw3_sl(0),
                    start=True, stop=False,
                )
                nc.tensor.matmul(ps[:], ones, db3_sl, start=False, stop=False)
                nc.tensor.matmul(
                    ps[:], d2_sb[1][:, ts(bi, 128)], dw3_sl(1),
                    start=False, stop=True,
                )
                nc.scalar.activation(st_["ob"][blk][:, bi, :], ps[:],
                                     AF.Sigmoid)

            # Software-pipelined emission: per-engine streams execute in
            # program order, so blk1's transposes/L1 are interleaved into
            # blk0's mid-layer chain to keep the PE busy while ACT/DVE work.
            transposes(0)
            _touch(nc, scratch, a_sb)
            for gj in range(4):
                l1(0, gj)
            _touch(nc, scratch, rest_sb)
            l2(0)
            l3(0)
            l4(0)
            transposes(1)
            l5(0, 0)
            l5(0, 1)
            for gj in range(4):
                l1(1, gj)
            l2(1)
            l3(1)
            l4(1)
            def store(blk, halves=False):
                # 5 loads + 3 stores fit the 8 HWDGE queues, so no DMA needs
                # a queue-reuse wait on top of its data wait (walrus allows
                # only one sync wait per DMA).  The final store is split so
                # the first half streams out while the last sigmoids run.
                ob = st_["ob"][blk]
                if halves:
                    nc.sync.dma_start(out=out_v[blk, :, 0:2], in_=ob[:, 0:2])
                    nc.sync.dma_start(out=out_v[blk, :, 2:4], in_=ob[:, 2:4])
                else:
                    nc.sync.dma_start(out=out_v[blk], in_=ob[:])

            l5(1, 0)
            l5(1, 1)
            for bi in range(4):
                l6(0, bi)
            store(0)
            for bi in range(4):
                l6(1, bi)
            store(1, halves=True)

    return nc


def _get_program():
    if "prog" not in _CACHE:
        _CACHE["prog"] = _build_program()
    return _CACHE["prog"]


def _pack_params(inputs):
    gw = np.asarray(inputs["gw"], dtype=np.float32)
    gb = np.asarray(inputs["gb"], dtype=np.float32)
    idx = np.asarray(inputs["idx"], dtype=np.int64)
    mask = np.asarray(inputs["mask"], dtype=np.float32)
    w1 = np.asarray(inputs["w1"], dtype=np.float32)
    b1 = np.asarray(inputs["b1"], dtype=np.float32)
    w2 = np.asarray(inputs["w2"], dtype=np.float32)
    b2 = np.asarray(inputs["b2"], dtype=np.float32)
    dw1 = np.asarray(inputs["dw1"], dtype=np.float32)
    db1 = np.asarray(inputs["db1"], dtype=np.float32)
    dw2 = np.asarray(inputs["dw2"], dtype=np.float32)
    db2 = np.asarray(inputs["db2"], dtype=np.float32)
    dw3 = np.asarray(inputs["dw3"], dtype=np.float32)
    db3 = np.asarray(inputs["db3"], dtype=np.float32)

    g, k = idx.shape
    assert g == G

    # Fold gather + grouped Dense(1) into a dense [S, GP] matrix.
    a_mat = np.zeros((S, GP), dtype=np.float32)
    gwm = (gw * mask).astype(np.float32)
    cols = np.repeat(np.arange(g, dtype=np.int64), k)
    np.add.at(a_mat, (idx.reshape(-1), cols), gwm.reshape(-1))

    bootpack = np.zeros((128, BOOT_COLS), dtype=np.float32)
    bootpack[:, IDENT_BASE : IDENT_BASE + 128] = np.eye(128, dtype=np.float32)
    gb_pad = np.zeros(GP, np.float32)
    gb_pad[:g] = gb
    for i in range(4):
        bootpack[:, BIAS_BASE + i] = gb_pad[i * 128 : (i + 1) * 128]
    bootpack[:, BIAS_BASE + 4] = b1
    bootpack[:LAT, BIAS_BASE + 5] = b2
    bootpack[:, BIAS_BASE + 6] = db1
    bootpack[:, BIAS_BASE + 7] = db2[:128]
    bootpack[:, BIAS_BASE + 8] = db2[128:]

    apack = np.zeros((128, A_COLS), dtype=np.float32)
    for fi in range(4):
        apack[:, fi * 512 : (fi + 1) * 512] = a_mat[fi * 128 : (fi + 1) * 128, :]

    restpack = np.zeros((128, REST_COLS), dtype=np.float32)
    w1_pad = np.zeros((GP, HID), dtype=np.float32)
    w1_pad[:g] = w1
    for fi in range(4):
        restpack[:, W1_BASE + fi * 128 : W1_BASE + (fi + 1) * 128] = \
            w1_pad[fi * 128 : (fi + 1) * 128, :]
    restpack[:, W2_BASE : W2_BASE + LAT] = w2
    restpack[:LAT, DW1_BASE : DW1_BASE + HID] = dw1
    restpack[:, DW2_BASE : DW2_BASE + HID2] = dw2
    for kj in range(2):
        restpack[:, DW3_BASE + kj * 512 : DW3_BASE + (kj + 1) * 512] = \
            dw3[kj * 128 : (kj + 1) * 128, :]
    restpack[0, ONES_BASE : ONES_BASE + 128] = 1.0
    restpack[0, DB3_BASE : DB3_BASE + S] = db3

    return bootpack, apack, restpack


def kernel(**inputs) -> np.ndarray:
    global last_results

    x = np.ascontiguousarray(np.asarray(inputs["x"], dtype=np.float32))
    assert x.shape == (B, S), x.shape
    bootpack, apack, restpack = _pack_params(inputs)

    in_maps = [
        {"x": x[c * BC : (c + 1) * BC], "bootpack": bootpack,
         "apack": apack, "restpack": restpack}
        for c in range(NCORES)
    ]

    nc = _get_program()
    trace = os.environ.get("KERNEL_TRACE", "0") == "1"
    res = run_bass_kernel_spmd(nc, in_maps, list(range(NCORES)), trace=trace)
    last_results = res
    out = np.concatenate([r["out"] for r in res.results], axis=0)
    return out.astype(np.float32)


if __name__ == "__main__":
    rng = np.random.RandomState(0)
    demo = {
        "x": rng.rand(B, S).astype(np.float32),
        "gw": rng.randn(G, 30).astype(np.float32),
        "gb": rng.randn(G).astype(np.float32) * 0.1,
        "idx": rng.randint(0, S, (G, 30)).astype(np.int32),
        "mask": (rng.rand(G, 30) > 0.5).astype(np.float32),
        "w1": rng.randn(G, HID).astype(np.float32) * 0.04,
        "b1": rng.randn(HID).astype(np.float32) * 0.1,
        "w2": rng.randn(HID, LAT).astype(np.float32) * 0.09,
        "b2": rng.randn(LAT).astype(np.float32) * 0.1,
        "dw1": rng.randn(LAT, HID).astype(np.float32) * 0.18,
        "db1": rng.randn(HID).astype(np.float32) * 0.1,
        "dw2": rng.randn(HID, HID2).astype(np.float32) * 0.09,
        "db2": rng.randn(HID2).astype(np.float32) * 0.1,
        "dw3": rng.randn(HID2, S).astype(np.float32) * 0.06,
        "db3": rng.randn(S).astype(np.float32) * 0.1,
    }
    out = kernel(**demo)
    print("out", out.shape, out.dtype, float(out.mean()))


# revision 50
# speedup vs baseline: 1.1790x; 1.0966x over previous
"""Trainium2 Bass kernel for the CustomAutoencoder problem.

Network (per batch row):
    h  = relu(einsum('gk,k->g', gw*mask, x[idx]) + gb)   # grouped gather-dense
    h  = relu(h @ w1 + b1); z = relu(h @ w2 + b2)
    d  = relu(z @ dw1 + db1); d = relu(d @ dw2 + db2)
    out = sigmoid(d @ dw3 + db3)

The gather+grouped-dense encoder is mathematically x @ A with
A[s, g] = sum_k (gw*mask)[g, k] * (idx[g, k] == s), so the whole model is a
dense MLP chain.  A is built on the host from the small param tensors
(replicated per the data-parallel sharding) and the batch dim is sharded
across 8 NeuronCores.

On-chip layout: activations are kept transposed [feature, batch]; x is
transposed on-device with PE transposes (f32r stream, 1.5 cyc/row).  All
matmul operands are bf16 (weights quantized on host, activation tiles
written bf16 by DVE/ACT) with fp32 PSUM accumulation; the rel-err budget
(2e-2) dwarfs the ~1e-3 this costs.  The last layer uses the transposed
activation as the stationary operand, yielding natural-layout output rows
directly; db3 enters as a K=1 ones-outer-product matmul.

DMA: input loads ride the Scalar (Activation) HWDGE queues in priority
order (x block 0 -> boot pack -> A pack -> rest pack -> row pack -> x
block 1); output stores ride the Sync queues one [128, 512] row-block at a
time so the tail overlaps the final sigmoids.  Each engine stays within
its 8 HWDGE queues so every DMA needs at most one sync wait (walrus
single-wait rule).  The tiny "touch" matmuls / scalar copies pre-advance
each engine's observed vector clock past DMA producers for the same
reason.
"""

import os
import sys

sys.path.insert(0, "/opt/trn_rl_repo")

import numpy as np
import ml_dtypes

import concourse.bass as bass
import concourse.tile as tile
from concourse import mybir
from concourse.bass import ts
from concourse.bass_utils import run_bass_kernel_spmd
from concourse.tile_rust import add_dep_helper

F32 = mybir.dt.float32
F32R = mybir.dt.float32r
BF16 = mybir.dt.bfloat16
AF = mybir.ActivationFunctionType
BF16_NP = ml_dtypes.bfloat16

B = 8192          # full batch
S = 512           # sample size (input/output features)
G = 510           # number of groups
GP = 512          # G padded to a multiple of 128
HID = 128
LAT = 32
HID2 = 256
NCORES = 8
BC = B // NCORES  # rows per core
BBLK = 512        # batch columns per block (PSUM free-dim max for fp32)
NBLK = BC // BBLK

# boot pack [128, BOOT_COLS] fp32: ident + per-partition biases.
IDENT_BASE = 0        # 128 x 128 fp32 identity (bitcast f32r at use)
BIAS_BASE = 128       # cols +0..3 gb chunks, +4 b1, +5 b2 (rows<32),
                      # +6 db1, +7..8 db2 chunks
BOOT_COLS = 137
# A pack [128, 2048] bf16: 4 x 512 (A[fi*128+p, g]).  bf16 matmul operands
# stream slightly slower than f32r when warm (258 vs 213 ns per 512 rows)
# but halve the LDWEIGHTS time and the PE data energy -- the cores run
# power-throttled (50% util cap), and the bf16 variant measures the least
# throttle-active time.  Mixed 32/16-bit operands are rejected by the
# verifier, so ALL matmul operands (weights + activations) are bf16; only
# the x transposes stay on the fp32 path.
A_COLS = 2048
# rest pack [128, REST_COLS] f32r (row 0 tail: ones row + db3, so the
# total DMA count stays at 8 = the HWDGE queue budget):
W1_BASE = 0           # 4 x 128  (w1[fi*128+p, m])
W2_BASE = 512         # 128 x 32
DW1_BASE = 544        # 32 x 128 (rows 0-31)
DW2_BASE = 672        # 128 x 256
DW3_BASE = 928        # 2 x 512  (dw3[kj*128+p, s])
ONES_BASE = 1952      # row 0, 128 cols of 1.0
DB3_BASE = 2080       # row 0, 512 cols
REST_COLS = 2592

_CACHE: dict = {}
last_results = None


def _touch(nc, scratch, tl):
    """1x1 PE matmul reading a corner of `tl`: advances the PE engine's
    observed vector clock past tl's producer (walrus S3_LW single-wait)."""
    return nc.tensor.matmul(
        scratch[0:1, 0:2], tl[0:1, 0:1], tl[0:1, 0:2], start=True, stop=True
    )


_STOUCH_IDX = [0]


def _stouch(nc, sdump, tl):
    """Scalar-engine equivalent of _touch for ACT-consumed (bias) tiles."""
    k = _STOUCH_IDX[0] % 32
    _STOUCH_IDX[0] += 1
    return nc.scalar.copy(out=sdump[0:1, k : k + 1], in_=tl[0:1, 0:1])


_VTOUCH_IDX = [0]


def _vtouch(nc, vdump, tl):
    """Vector-engine equivalent of _touch for DVE-consumed tiles."""
    k = _VTOUCH_IDX[0] % 32
    _VTOUCH_IDX[0] += 1
    return nc.vector.tensor_copy(vdump[0:1, k : k + 1], tl[0:1, 0:1])


class SplitDrainTileContext(tile.TileContext):
    """TileContext whose kernel-tail drain carries at most one sync wait per
    instruction: this walrus build rejects >1 sync wait on any instruction,
    and the stock tail drain aggregates one wait per active proc."""

    def _drain_and_barrier(self, tick_clock, wait_clock):
        from concourse.vector_clock import ScopedClock, VectorClock

        gc = tick_clock.global_clock
        n = len(gc)
        for p in range(n):
            t = gc[p]
            if t == 0:
                continue
            single = [0] * n
            single[p] = t
            nop = self.nc.sync.nop(nofuse=True, hint="split_drain_wait")
            wait_clock.add_sem_waits(
                nop.ins, ScopedClock({None: VectorClock(single)})
            )
        # The per-proc nops above already enforce every outstanding tick in
        # SP program order, so the drain itself needs no waits.
        self.nc.sync.drain()
        self.nc.all_engine_barrier()
        assert self.sems is not None
        popped = self.nc._tile_sem_poison_stack.pop()
        assert popped is self._sem_poison
        self.nc.clear_and_free_semaphores(list(self.sems.allocated().values()))
        self.nc.all_engine_barrier()


def _build_program():
    nc = bass.Bass()

    # x stays plain fp32: fp32-input PE transposes measure FASTER than the
    # f32r variant (LDWEIGHTS ~74 ns vs ~330 ns per 128x128 chunk) and keep
    # the PE stream dense through the warm-up window.
    x_d = nc.declare_dram_parameter("x", [BC, S], F32, isOutput=False)
    boot_d = nc.declare_dram_parameter("bootpack", [128, BOOT_COLS], F32R,
                                       isOutput=False)
    a_d = nc.declare_dram_parameter("apack", [128, A_COLS], BF16,
                                    isOutput=False)
    rest_d = nc.declare_dram_parameter("restpack", [128, REST_COLS], BF16,
                                       isOutput=False)
    out_d = nc.declare_dram_parameter("out", [BC, S], F32, isOutput=True)

    x_v = x_d.rearrange("(k i p) s -> k p i s", p=128, i=4)    # [NBLK,128,4,512]
    out_v = out_d.rearrange("(k i p) s -> k p i s", p=128, i=4)

    with SplitDrainTileContext(nc) as tc:
        with (
            tc.tile_pool(name="weights", bufs=1) as wp,
            tc.tile_pool(name="xin", bufs=2) as xp,
            tc.tile_pool(name="xt", bufs=8) as xtp,
            tc.tile_pool(name="acts", bufs=4) as ap_,
            tc.tile_pool(name="h1", bufs=8) as h1p,
            tc.tile_pool(name="outs", bufs=2) as op_,
            tc.tile_pool(name="pt", bufs=2, space="PSUM") as ptp,
            tc.tile_pool(name="p1", bufs=3, space="PSUM") as p1p,
            tc.tile_pool(name="pmid", bufs=2, space="PSUM") as pmp,
            tc.tile_pool(name="psc", bufs=1, space="PSUM") as pscp,
        ):
            # l6 ping-pongs on the pt pool (free once the transposes are
            # done), so p1 gets a third bank to decouple the L1 matmuls from
            # the DVE h1 read-out.
            scratch = pscp.tile([1, 2], F32)
            sdump = wp.tile([1, 32], F32, tag="sdump")
            vdump = wp.tile([1, 32], F32, tag="vdump")
            _STOUCH_IDX[0] = 0
            _VTOUCH_IDX[0] = 0

            # Load priority: x block 0 gates the first transposes, then the
            # small boot pack (ident + biases), then A (gates L1).
            xbs = []
            xb = xp.tile([128, 4, BBLK], F32, tag="xb")
            nc.scalar.dma_start(out=xb[:], in_=x_v[0])
            xbs.append(xb)
            boot_sb = wp.tile([128, BOOT_COLS], F32R, tag="bootpack")
            nc.scalar.dma_start(out=boot_sb[:], in_=boot_d[:, :])
            _touch(nc, scratch, boot_sb)
            _stouch(nc, sdump, boot_sb[:, 0:1].bitcast(F32))
            _vtouch(nc, vdump, boot_sb[:, 0:1].bitcast(F32))
            a_sb = wp.tile([128, A_COLS], BF16, tag="apack")
            nc.scalar.dma_start(out=a_sb[:], in_=a_d[:, :])
            rest_sb = wp.tile([128, REST_COLS], BF16, tag="restpack")
            nc.scalar.dma_start(out=rest_sb[:], in_=rest_d[:, :])
            xb = xp.tile([128, 4, BBLK], F32, tag="xb")
            nc.scalar.dma_start(out=xb[:], in_=x_v[1])
            xbs.append(xb)

            ident = boot_sb[:, IDENT_BASE : IDENT_BASE + 128].bitcast(F32)
            ones = rest_sb[0:1, ONES_BASE : ONES_BASE + 128]
            db3_sl = rest_sb[0:1, DB3_BASE : DB3_BASE + S]   # rhs [1, 512]

            def a_sl(fi, gj):        # lhsT [128, 128]
                c = fi * 512 + gj * 128
                return a_sb[:, c : c + 128]

            def w1_sl(fi):           # lhsT [128, 128]
                return rest_sb[:, W1_BASE + fi * 128 : W1_BASE + (fi + 1) * 128]

            w2_sl = rest_sb[:, W2_BASE : W2_BASE + LAT]          # [128, 32]
            dw1_sl = rest_sb[0:LAT, DW1_BASE : DW1_BASE + HID]   # [32, 128]

            def dw2_sl(j):           # lhsT [128, 128]
                return rest_sb[:, DW2_BASE + j * 128 : DW2_BASE + (j + 1) * 128]

            def dw3_sl(kj):          # rhs [128, 512]
                return rest_sb[:, DW3_BASE + kj * 512 : DW3_BASE + (kj + 1) * 512]

            def bias_col(i, rows=128):
                return boot_sb[0:rows, BIAS_BASE + i : BIAS_BASE + i + 1].bitcast(F32)

            gb_b = [bias_col(i) for i in range(4)]
            b1_b = bias_col(4)
            b2_b = bias_col(5, rows=LAT)
            db1_b = bias_col(6)
            db2_b = [bias_col(7 + j) for j in range(2)]

            st_ = {"xt": {}, "h1": {}, "h2": {}, "z": {}, "d1": {}, "d2": {},
                   "ob": {}}

            def transposes(blk):
                xbr = xbs[blk]
                xtch = _touch(nc, scratch, xbr[:, 0, :])
                xt_sb = []
                for fj in range(4):
                    pt = ptp.tile([128, BBLK], F32)
                    for bi in range(4):
                        tp = nc.tensor.transpose(
                            pt[:, ts(bi, 128)], xbr[:, bi, ts(fj, 128)], ident
                        )
                        if bi == 0:
                            add_dep_helper(tp.ins, xtch.ins, sync=False,
                                           reason="transpose after x touch")
                    st = xtp.tile([128, BBLK], BF16)
                    nc.vector.tensor_copy(st[:], pt[:])
                    # PE observes the DVE tick so the next transpose group
                    # reusing this PSUM slot needs at most one sync wait.
                    _touch(nc, scratch, st)
                    xt_sb.append(st)
                st_["xt"][blk] = xt_sb

            def l1(blk, gj):
                # h1T[gj] = relu(sum_fi A[fi, gj].T @ xT[fi] + gb[gj])
                ps = p1p.tile([128, BBLK], F32, tag="p1")
                for fi in range(4):
                    nc.tensor.matmul(
                        ps[:], a_sl(fi, gj), st_["xt"][blk][fi][:],
                        start=(fi == 0), stop=(fi == 3),
                    )
                h = h1p.tile([128, BBLK], BF16)
                nc.vector.tensor_scalar(
                    h[:], ps[:], gb_b[gj], 0.0,
                    op0=mybir.AluOpType.add, op1=mybir.AluOpType.max,
                )
                st_["h1"].setdefault(blk, []).append(h)
                if gj == 3:
                    # PE observes the DVE ticks of the h1 producers, so L2/L6
                    # matmuls keep at most one sync wait.
                    _touch(nc, scratch, h)

            def l2(blk):
                ps = pmp.tile([128, BBLK], F32, tag="pmid")
                for fi in range(4):
                    nc.tensor.matmul(
                        ps[:], w1_sl(fi), st_["h1"][blk][fi][:],
                        start=(fi == 0), stop=(fi == 3),
                    )
                h2 = ap_.tile([HID, BBLK], BF16, tag="h2")
                nc.scalar.activation(h2[:], ps[:], AF.Relu, bias=b1_b)
                st_["h2"][blk] = h2

            def l3(blk):
                ps = pmp.tile([LAT, BBLK], F32, tag="pmid")
                nc.tensor.matmul(ps[:], w2_sl, st_["h2"][blk][:], start=True,
                                 stop=True)
                z = ap_.tile([LAT, BBLK], BF16, tag="z")
                nc.scalar.activation(z[:], ps[:], AF.Relu, bias=b2_b)
                st_["z"][blk] = z

            def l4(blk):
                ps = pmp.tile([HID, BBLK], F32, tag="pmid")
                nc.tensor.matmul(ps[:], dw1_sl, st_["z"][blk][:], start=True,
                                 stop=True)
                d1 = ap_.tile([HID, BBLK], BF16, tag="d1")
                nc.scalar.activation(d1[:], ps[:], AF.Relu, bias=db1_b)
                st_["d1"][blk] = d1

            def l5(blk, j):
                ps = pmp.tile([128, BBLK], F32, tag="pmid")
                nc.tensor.matmul(ps[:], dw2_sl(j), st_["d1"][blk][:],
                                 start=True, stop=True)
                d2 = ap_.tile([128, BBLK], BF16, tag=f"d2_{j}")
                nc.scalar.activation(d2[:], ps[:], AF.Relu, bias=db2_b[j])
                st_["d2"].setdefault(blk, []).append(d2)

            def l6(blk, bi):
                # out[bi] = sigmoid(sum_j d2T[j][:, bi].T @ dw3[j] + db3),
                # natural layout directly; db3 enters as a K=1 ones-outer-
                # product matmul inside the accumulation group.
                if bi == 0:
                    ob = op_.tile([128, 4, S], F32, tag="ob")
                    st_["ob"][blk] = ob
                d2_sb = st_["d2"][blk]
                ps = ptp.tile([128, S], F32, tag="pt")
                nc.tensor.matmul(
                    ps[:], d2_sb[0][:, ts(bi, 128)], dw3_sl(0),
                    start=True, stop=False,
                )
                nc.tensor.matmul(ps[:], ones, db3_sl, start=False, stop=False)
                nc.tensor.matmul(
                    ps[:], d2_sb[1][:, ts(bi, 128)], dw3_sl(1),
                    start=False, stop=True,
                )
                nc.scalar.activation(st_["ob"][blk][:, bi, :], ps[:],
                                     AF.Sigmoid)

            # Software-pipelined emission: per-engine streams execute in
            # program order, so blk1's transposes/L1 are interleaved into
            # blk0's mid-layer chain to keep the PE busy while ACT/DVE work.
            transposes(0)
            _touch(nc, scratch, a_sb)
            for gj in range(4):
                l1(0, gj)
            _touch(nc, scratch, rest_sb)
            l2(0)
            l3(0)
            l4(0)
            transposes(1)
            l5(0, 0)
            l5(0, 1)
            for gj in range(4):
                l1(1, gj)
            l2(1)
            l3(1)
            l4(1)
            def store(blk, halves=False):
                # 5 loads + 3 stores fit the 8 HWDGE queues, so no DMA needs
                # a queue-reuse wait on top of its data wait (walrus allows
                # only one sync wait per DMA).  The final store is split so
                # the first half streams out while the last sigmoids run.
                ob = st_["ob"][blk]
                if halves:
                    nc.sync.dma_start(out=out_v[blk, :, 0:2], in_=ob[:, 0:2])
                    nc.sync.dma_start(out=out_v[blk, :, 2:4], in_=ob[:, 2:4])
                else:
                    nc.sync.dma_start(out=out_v[blk], in_=ob[:])

            l5(1, 0)
            l5(1, 1)
            for bi in range(4):
                l6(0, bi)
            store(0)
            for bi in range(4):
                l6(1, bi)
            store(1, halves=True)

    return nc


def _get_program():
    if "prog" not in _CACHE:
        _CACHE["prog"] = _build_program()
    return _CACHE["prog"]


def _pack_params(inputs):
    gw = np.asarray(inputs["gw"], dtype=np.float32)
    gb = np.asarray(inputs["gb"], dtype=np.float32)
    idx = np.asarray(inputs["idx"], dtype=np.int64)
    mask = np.asarray(inputs["mask"], dtype=np.float32)
    w1 = np.asarray(inputs["w1"], dtype=np.float32)
    b1 = np.asarray(inputs["b1"], dtype=np.float32)
    w2 = np.asarray(inputs["w2"], dtype=np.float32)
    b2 = np.asarray(inputs["b2"], dtype=np.float32)
    dw1 = np.asarray(inputs["dw1"], dtype=np.float32)
    db1 = np.asarray(inputs["db1"], dtype=np.float32)
    dw2 = np.asarray(inputs["dw2"], dtype=np.float32)
    db2 = np.asarray(inputs["db2"], dtype=np.float32)
    dw3 = np.asarray(inputs["dw3"], dtype=np.float32)
    db3 = np.asarray(inputs["db3"], dtype=np.float32)

    g, k = idx.shape
    assert g == G

    # Fold gather + grouped Dense(1) into a dense [S, GP] matrix.
    a_mat = np.zeros((S, GP), dtype=np.float32)
    gwm = (gw * mask).astype(np.float32)
    cols = np.repeat(np.arange(g, dtype=np.int64), k)
    np.add.at(a_mat, (idx.reshape(-1), cols), gwm.reshape(-1))

    bootpack = np.zeros((128, BOOT_COLS), dtype=np.float32)
    bootpack[:, IDENT_BASE : IDENT_BASE + 128] = np.eye(128, dtype=np.float32)
    gb_pad = np.zeros(GP, np.float32)
    gb_pad[:g] = gb
    for i in range(4):
        bootpack[:, BIAS_BASE + i] = gb_pad[i * 128 : (i + 1) * 128]
    bootpack[:, BIAS_BASE + 4] = b1
    bootpack[:LAT, BIAS_BASE + 5] = b2
    bootpack[:, BIAS_BASE + 6] = db1
    bootpack[:, BIAS_BASE + 7] = db2[:128]
    bootpack[:, BIAS_BASE + 8] = db2[128:]

    apack = np.zeros((128, A_COLS), dtype=BF16_NP)
    for fi in range(4):
        apack[:, fi * 512 : (fi + 1) * 512] = \
            a_mat[fi * 128 : (fi + 1) * 128, :].astype(BF16_NP)

    restpack = np.zeros((128, REST_COLS), dtype=BF16_NP)
    w1_pad = np.zeros((GP, HID), dtype=np.float32)
    w1_pad[:g] = w1
    for fi in range(4):
        restpack[:, W1_BASE + fi * 128 : W1_BASE + (fi + 1) * 128] = \
            w1_pad[fi * 128 : (fi + 1) * 128, :].astype(BF16_NP)
    restpack[:, W2_BASE : W2_BASE + LAT] = w2.astype(BF16_NP)
    restpack[:LAT, DW1_BASE : DW1_BASE + HID] = dw1.astype(BF16_NP)
    restpack[:, DW2_BASE : DW2_BASE + HID2] = dw2.astype(BF16_NP)
    for kj in range(2):
        restpack[:, DW3_BASE + kj * 512 : DW3_BASE + (kj + 1) * 512] = \
            dw3[kj * 128 : (kj + 1) * 128, :].astype(BF16_NP)
    restpack[0, ONES_BASE : ONES_BASE + 128] = 1.0
    restpack[0, DB3_BASE : DB3_BASE + S] = db3.astype(BF16_NP)

    return bootpack, apack, restpack


def kernel(**inputs) -> np.ndarray:
    global last_results

    x = np.ascontiguousarray(np.asarray(inputs["x"], dtype=np.float32))
    assert x.shape == (B, S), x.shape
    bootpack, apack, restpack = _pack_params(inputs)

    in_maps = [
        {"x": x[c * BC : (c + 1) * BC], "bootpack": bootpack,
         "apack": apack, "restpack": restpack}
        for c in range(NCORES)
    ]

    nc = _get_program()
    trace = os.environ.get("KERNEL_TRACE", "0") == "1"
    res = run_bass_kernel_spmd(nc, in_maps, list(range(NCORES)), trace=trace)
    last_results = res
    out = np.concatenate([r["out"] for r in res.results], axis=0)
    return out.astype(np.float32)


if __name__ == "__main__":
    rng = np.random.RandomState(0)
    demo = {
        "x": rng.rand(B, S).astype(np.float32),
        "gw": rng.randn(G, 30).astype(np.float32),
        "gb": rng.randn(G).astype(np.float32) * 0.1,
        "idx": rng.randint(0, S, (G, 30)).astype(np.int32),
        "mask": (rng.rand(G, 30) > 0.5).astype(np.float32),
        "w1": rng.randn(G, HID).astype(np.float32) * 0.04,
        "b1": rng.randn(HID).astype(np.float32) * 0.1,
        "w2": rng.randn(HID, LAT).astype(np.float32) * 0.09,
        "b2": rng.randn(LAT).astype(np.float32) * 0.1,
        "dw1": rng.randn(LAT, HID).astype(np.float32) * 0.18,
        "db1": rng.randn(HID).astype(np.float32) * 0.1,
        "dw2": rng.randn(HID, HID2).astype(np.float32) * 0.09,
        "db2": rng.randn(HID2).astype(np.float32) * 0.1,
        "dw3": rng.randn(HID2, S).astype(np.float32) * 0.06,
        "db3": rng.randn(S).astype(np.float32) * 0.1,
    }
    out = kernel(**demo)
    print("out", out.shape, out.dtype, float(out.mean()))


# revision 51
# speedup vs baseline: 1.2230x; 1.0373x over previous
"""Trainium2 Bass kernel for the CustomAutoencoder problem.

Network (per batch row):
    h  = relu(einsum('gk,k->g', gw*mask, x[idx]) + gb)   # grouped gather-dense
    h  = relu(h @ w1 + b1); z = relu(h @ w2 + b2)
    d  = relu(z @ dw1 + db1); d = relu(d @ dw2 + db2)
    out = sigmoid(d @ dw3 + db3)

The gather+grouped-dense encoder is mathematically x @ A with
A[s, g] = sum_k (gw*mask)[g, k] * (idx[g, k] == s), so the whole model is a
dense MLP chain.  A is built on the host from the small param tensors
(replicated per the data-parallel sharding) and the batch dim is sharded
across 8 NeuronCores.

On-chip layout: activations are kept transposed [feature, batch]; x is
transposed on-device with PE transposes (f32r stream, 1.5 cyc/row).  All
matmul operands are bf16 (weights quantized on host, activation tiles
written bf16 by DVE/ACT) with fp32 PSUM accumulation; the rel-err budget
(2e-2) dwarfs the ~1e-3 this costs.  The last layer uses the transposed
activation as the stationary operand, yielding natural-layout output rows
directly; db3 enters as a K=1 ones-outer-product matmul.

DMA: input loads ride the Scalar (Activation) HWDGE queues in priority
order (x block 0 -> boot pack -> A pack -> rest pack -> row pack -> x
block 1); output stores ride the Sync queues one [128, 512] row-block at a
time so the tail overlaps the final sigmoids.  Each engine stays within
its 8 HWDGE queues so every DMA needs at most one sync wait (walrus
single-wait rule).  The tiny "touch" matmuls / scalar copies pre-advance
each engine's observed vector clock past DMA producers for the same
reason.
"""

import os
import sys

sys.path.insert(0, "/opt/trn_rl_repo")

import numpy as np
import ml_dtypes

import concourse.bass as bass
import concourse.bass_utils as _bass_utils
import concourse.tile as tile

# The NEFF epilogue resets every physical semaphore walrus allocated, one
# engine instruction per semaphore (~285 sems, ~9.5 us of measured kernel
# time).  Capping the allocator forces ID reuse and shrinks that epilogue.
_MAX_SEMS = os.environ.get("KERNEL_MAX_SEMS", "")
if _MAX_SEMS and not getattr(_bass_utils, "_kernel_sem_patch", False):
    _orig_walrus_args = _bass_utils.get_walrus_args

    def _patched_walrus_args(*a, **k):
        return _orig_walrus_args(*a, **k) + [f"--max-sem-num={_MAX_SEMS}"]

    _bass_utils.get_walrus_args = _patched_walrus_args
    _bass_utils._kernel_sem_patch = True
from concourse import mybir
from concourse.bass import ts
from concourse.bass_utils import run_bass_kernel_spmd
from concourse.tile_rust import add_dep_helper

F32 = mybir.dt.float32
F32R = mybir.dt.float32r
BF16 = mybir.dt.bfloat16
AF = mybir.ActivationFunctionType
BF16_NP = ml_dtypes.bfloat16

B = 8192          # full batch
S = 512           # sample size (input/output features)
G = 510           # number of groups
GP = 512          # G padded to a multiple of 128
HID = 128
LAT = 32
HID2 = 256
NCORES = 8
BC = B // NCORES  # rows per core
BBLK = 512        # batch columns per block (PSUM free-dim max for fp32)
NBLK = BC // BBLK

# boot pack [128, BOOT_COLS] fp32: ident + per-partition biases.
IDENT_BASE = 0        # 128 x 128 fp32 identity (bitcast f32r at use)
BIAS_BASE = 128       # cols +0..3 gb chunks, +4 b1, +5 b2 (rows<32),
                      # +6 db1, +7..8 db2 chunks
BOOT_COLS = 137
# A pack [128, 2048] bf16: 4 x 512 (A[fi*128+p, g]).  bf16 matmul operands
# stream slightly slower than f32r when warm (258 vs 213 ns per 512 rows)
# but halve the LDWEIGHTS time and the PE data energy -- the cores run
# power-throttled (50% util cap), and the bf16 variant measures the least
# throttle-active time.  Mixed 32/16-bit operands are rejected by the
# verifier, so ALL matmul operands (weights + activations) are bf16; only
# the x transposes stay on the fp32 path.
A_COLS = 2048
# rest pack [128, REST_COLS] f32r (row 0 tail: ones row + db3, so the
# total DMA count stays at 8 = the HWDGE queue budget):
W1_BASE = 0           # 4 x 128  (w1[fi*128+p, m])
W2_BASE = 512         # 128 x 32
DW1_BASE = 544        # 32 x 128 (rows 0-31)
DW2_BASE = 672        # 128 x 256
DW3_BASE = 928        # 2 x 512  (dw3[kj*128+p, s])
ONES_BASE = 1952      # row 0, 128 cols of 1.0
DB3_BASE = 2080       # row 0, 512 cols
REST_COLS = 2592

_CACHE: dict = {}
last_results = None


def _touch(nc, scratch, tl):
    """1x1 PE matmul reading a corner of `tl`: advances the PE engine's
    observed vector clock past tl's producer (walrus S3_LW single-wait)."""
    return nc.tensor.matmul(
        scratch[0:1, 0:2], tl[0:1, 0:1], tl[0:1, 0:2], start=True, stop=True
    )


_STOUCH_IDX = [0]


def _stouch(nc, sdump, tl):
    """Scalar-engine equivalent of _touch for ACT-consumed (bias) tiles."""
    k = _STOUCH_IDX[0] % 32
    _STOUCH_IDX[0] += 1
    return nc.scalar.copy(out=sdump[0:1, k : k + 1], in_=tl[0:1, 0:1])


_VTOUCH_IDX = [0]


def _vtouch(nc, vdump, tl):
    """Vector-engine equivalent of _touch for DVE-consumed tiles."""
    k = _VTOUCH_IDX[0] % 32
    _VTOUCH_IDX[0] += 1
    return nc.vector.tensor_copy(vdump[0:1, k : k + 1], tl[0:1, 0:1])


class SplitDrainTileContext(tile.TileContext):
    """TileContext whose kernel-tail drain carries at most one sync wait per
    instruction: this walrus build rejects >1 sync wait on any instruction,
    and the stock tail drain aggregates one wait per active proc."""

    def _drain_and_barrier(self, tick_clock, wait_clock):
        from concourse.vector_clock import ScopedClock, VectorClock

        gc = tick_clock.global_clock
        n = len(gc)
        for p in range(n):
            t = gc[p]
            if t == 0:
                continue
            single = [0] * n
            single[p] = t
            nop = self.nc.sync.nop(nofuse=True, hint="split_drain_wait")
            wait_clock.add_sem_waits(
                nop.ins, ScopedClock({None: VectorClock(single)})
            )
        # The per-proc nops above already enforce every outstanding tick in
        # SP program order, so the drain itself needs no waits.
        self.nc.sync.drain()
        self.nc.all_engine_barrier()
        assert self.sems is not None
        popped = self.nc._tile_sem_poison_stack.pop()
        assert popped is self._sem_poison
        self.nc.clear_and_free_semaphores(list(self.sems.allocated().values()))
        self.nc.all_engine_barrier()


def _build_program():
    nc = bass.Bass()

    # x stays plain fp32: fp32-input PE transposes measure FASTER than the
    # f32r variant (LDWEIGHTS ~74 ns vs ~330 ns per 128x128 chunk) and keep
    # the PE stream dense through the warm-up window.
    x_d = nc.declare_dram_parameter("x", [BC, S], F32, isOutput=False)
    boot_d = nc.declare_dram_parameter("bootpack", [128, BOOT_COLS], F32R,
                                       isOutput=False)
    a_d = nc.declare_dram_parameter("apack", [128, A_COLS], BF16,
                                    isOutput=False)
    rest_d = nc.declare_dram_parameter("restpack", [128, REST_COLS], BF16,
                                       isOutput=False)
    out_d = nc.declare_dram_parameter("out", [BC, S], F32, isOutput=True)

    x_v = x_d.rearrange("(k i p) s -> k p i s", p=128, i=4)    # [NBLK,128,4,512]
    out_v = out_d.rearrange("(k i p) s -> k p i s", p=128, i=4)

    with SplitDrainTileContext(nc) as tc:
        with (
            tc.tile_pool(name="weights", bufs=1) as wp,
            tc.tile_pool(name="xin", bufs=2) as xp,
            tc.tile_pool(name="xt", bufs=8) as xtp,
            tc.tile_pool(name="acts", bufs=4) as ap_,
            tc.tile_pool(name="h1", bufs=8) as h1p,
            tc.tile_pool(name="outs", bufs=2) as op_,
            tc.tile_pool(name="pt", bufs=2, space="PSUM") as ptp,
            tc.tile_pool(name="p1", bufs=3, space="PSUM") as p1p,
            tc.tile_pool(name="pmid", bufs=2, space="PSUM") as pmp,
            tc.tile_pool(name="psc", bufs=1, space="PSUM") as pscp,
        ):
            # l6 ping-pongs on the pt pool (free once the transposes are
            # done), so p1 gets a third bank to decouple the L1 matmuls from
            # the DVE h1 read-out.
            scratch = pscp.tile([1, 2], F32)
            sdump = wp.tile([1, 32], F32, tag="sdump")
            vdump = wp.tile([1, 32], F32, tag="vdump")
            _STOUCH_IDX[0] = 0
            _VTOUCH_IDX[0] = 0

            # Load priority: x block 0 gates the first transposes, then the
            # small boot pack (ident + biases), then A (gates L1).
            xbs = []
            xb = xp.tile([128, 4, BBLK], F32, tag="xb")
            nc.scalar.dma_start(out=xb[:], in_=x_v[0])
            xbs.append(xb)
            boot_sb = wp.tile([128, BOOT_COLS], F32R, tag="bootpack")
            nc.scalar.dma_start(out=boot_sb[:], in_=boot_d[:, :])
            _touch(nc, scratch, boot_sb)
            _stouch(nc, sdump, boot_sb[:, 0:1].bitcast(F32))
            _vtouch(nc, vdump, boot_sb[:, 0:1].bitcast(F32))
            a_sb = wp.tile([128, A_COLS], BF16, tag="apack")
            nc.scalar.dma_start(out=a_sb[:], in_=a_d[:, :])
            rest_sb = wp.tile([128, REST_COLS], BF16, tag="restpack")
            nc.scalar.dma_start(out=rest_sb[:], in_=rest_d[:, :])
            xb = xp.tile([128, 4, BBLK], F32, tag="xb")
            nc.scalar.dma_start(out=xb[:], in_=x_v[1])
            xbs.append(xb)

            ident = boot_sb[:, IDENT_BASE : IDENT_BASE + 128].bitcast(F32)
            ones = rest_sb[0:1, ONES_BASE : ONES_BASE + 128]
            db3_sl = rest_sb[0:1, DB3_BASE : DB3_BASE + S]   # rhs [1, 512]

            def a_sl(fi, gj):        # lhsT [128, 128]
                c = fi * 512 + gj * 128
                return a_sb[:, c : c + 128]

            def w1_sl(fi):           # lhsT [128, 128]
                return rest_sb[:, W1_BASE + fi * 128 : W1_BASE + (fi + 1) * 128]

            w2_sl = rest_sb[:, W2_BASE : W2_BASE + LAT]          # [128, 32]
            dw1_sl = rest_sb[0:LAT, DW1_BASE : DW1_BASE + HID]   # [32, 128]

            def dw2_sl(j):           # lhsT [128, 128]
                return rest_sb[:, DW2_BASE + j * 128 : DW2_BASE + (j + 1) * 128]

            def dw3_sl(kj):          # rhs [128, 512]
                return rest_sb[:, DW3_BASE + kj * 512 : DW3_BASE + (kj + 1) * 512]

            def bias_col(i, rows=128):
                return boot_sb[0:rows, BIAS_BASE + i : BIAS_BASE + i + 1].bitcast(F32)

            gb_b = [bias_col(i) for i in range(4)]
            b1_b = bias_col(4)
            b2_b = bias_col(5, rows=LAT)
            db1_b = bias_col(6)
            db2_b = [bias_col(7 + j) for j in range(2)]

            st_ = {"xt": {}, "h1": {}, "h2": {}, "z": {}, "d1": {}, "d2": {},
                   "ob": {}}

            def transposes(blk):
                xbr = xbs[blk]
                xtch = _touch(nc, scratch, xbr[:, 0, :])
                xt_sb = []
                for fj in range(4):
                    pt = ptp.tile([128, BBLK], F32)
                    for bi in range(4):
                        tp = nc.tensor.transpose(
                            pt[:, ts(bi, 128)], xbr[:, bi, ts(fj, 128)], ident
                        )
                        if bi == 0:
                            add_dep_helper(tp.ins, xtch.ins, sync=False,
                                           reason="transpose after x touch")
                    st = xtp.tile([128, BBLK], BF16)
                    nc.vector.tensor_copy(st[:], pt[:])
                    # PE observes the DVE tick so the next transpose group
                    # reusing this PSUM slot needs at most one sync wait.
                    _touch(nc, scratch, st)
                    xt_sb.append(st)
                st_["xt"][blk] = xt_sb

            def l1(blk, gj):
                # h1T[gj] = relu(sum_fi A[fi, gj].T @ xT[fi] + gb[gj])
                ps = p1p.tile([128, BBLK], F32, tag="p1")
                for fi in range(4):
                    nc.tensor.matmul(
                        ps[:], a_sl(fi, gj), st_["xt"][blk][fi][:],
                        start=(fi == 0), stop=(fi == 3),
                    )
                h = h1p.tile([128, BBLK], BF16)
                nc.vector.tensor_scalar(
                    h[:], ps[:], gb_b[gj], 0.0,
                    op0=mybir.AluOpType.add, op1=mybir.AluOpType.max,
                )
                st_["h1"].setdefault(blk, []).append(h)
                if gj == 3:
                    # PE observes the DVE ticks of the h1 producers, so L2/L6
                    # matmuls keep at most one sync wait.
                    _touch(nc, scratch, h)

            def l2(blk):
                ps = pmp.tile([128, BBLK], F32, tag="pmid")
                for fi in range(4):
                    nc.tensor.matmul(
                        ps[:], w1_sl(fi), st_["h1"][blk][fi][:],
                        start=(fi == 0), stop=(fi == 3),
                    )
                h2 = ap_.tile([HID, BBLK], BF16, tag="h2")
                nc.scalar.activation(h2[:], ps[:], AF.Relu, bias=b1_b)
                st_["h2"][blk] = h2

            def l3(blk):
                ps = pmp.tile([LAT, BBLK], F32, tag="pmid")
                nc.tensor.matmul(ps[:], w2_sl, st_["h2"][blk][:], start=True,
                                 stop=True)
                z = ap_.tile([LAT, BBLK], BF16, tag="z")
                nc.scalar.activation(z[:], ps[:], AF.Relu, bias=b2_b)
                st_["z"][blk] = z

            def l4(blk):
                ps = pmp.tile([HID, BBLK], F32, tag="pmid")
                nc.tensor.matmul(ps[:], dw1_sl, st_["z"][blk][:], start=True,
                                 stop=True)
                d1 = ap_.tile([HID, BBLK], BF16, tag="d1")
                nc.scalar.activation(d1[:], ps[:], AF.Relu, bias=db1_b)
                st_["d1"][blk] = d1

            def l5(blk, j):
                ps = pmp.tile([128, BBLK], F32, tag="pmid")
                nc.tensor.matmul(ps[:], dw2_sl(j), st_["d1"][blk][:],
                                 start=True, stop=True)
                d2 = ap_.tile([128, BBLK], BF16, tag=f"d2_{j}")
                nc.scalar.activation(d2[:], ps[:], AF.Relu, bias=db2_b[j])
                st_["d2"].setdefault(blk, []).append(d2)

            def l6(blk, bi):
                # out[bi] = sigmoid(sum_j d2T[j][:, bi].T @ dw3[j] + db3),
                # natural layout directly; db3 enters as a K=1 ones-outer-
                # product matmul inside the accumulation group.
                if bi == 0:
                    ob = op_.tile([128, 4, S], F32, tag="ob")
                    st_["ob"][blk] = ob
                d2_sb = st_["d2"][blk]
                ps = ptp.tile([128, S], F32, tag="pt")
                nc.tensor.matmul(
                    ps[:], d2_sb[0][:, ts(bi, 128)], dw3_sl(0),
                    start=True, stop=False,
                )
                nc.tensor.matmul(ps[:], ones, db3_sl, start=False, stop=False)
                nc.tensor.matmul(
                    ps[:], d2_sb[1][:, ts(bi, 128)], dw3_sl(1),
                    start=False, stop=True,
                )
                nc.scalar.activation(st_["ob"][blk][:, bi, :], ps[:],
                                     AF.Sigmoid)

            # Software-pipelined emission: per-engine streams execute in
            # program order, so blk1's transposes/L1 are interleaved into
            # blk0's mid-layer chain to keep the PE busy while ACT/DVE work.
            transposes(0)
            _touch(nc, scratch, a_sb)
            for gj in range(4):
                l1(0, gj)
            _touch(nc, scratch, rest_sb)
            l2(0)
            l3(0)
            l4(0)
            transposes(1)
            l5(0, 0)
            l5(0, 1)
            for gj in range(4):
                l1(1, gj)
            l2(1)
            l3(1)
            l4(1)
            def store(blk, halves=False):
                # 5 loads + 3 stores fit the 8 HWDGE queues, so no DMA needs
                # a queue-reuse wait on top of its data wait (walrus allows
                # only one sync wait per DMA).  The final store is split so
                # the first half streams out while the last sigmoids run.
                ob = st_["ob"][blk]
                if halves:
                    nc.sync.dma_start(out=out_v[blk, :, 0:2], in_=ob[:, 0:2])
                    nc.sync.dma_start(out=out_v[blk, :, 2:4], in_=ob[:, 2:4])
                else:
                    nc.sync.dma_start(out=out_v[blk], in_=ob[:])

            l5(1, 0)
            l5(1, 1)
            for bi in range(4):
                l6(0, bi)
            store(0)
            for bi in range(4):
                l6(1, bi)
            store(1, halves=True)

    return nc


def _get_program():
    if "prog" not in _CACHE:
        _CACHE["prog"] = _build_program()
    return _CACHE["prog"]


def _pack_params(inputs):
    gw = np.asarray(inputs["gw"], dtype=np.float32)
    gb = np.asarray(inputs["gb"], dtype=np.float32)
    idx = np.asarray(inputs["idx"], dtype=np.int64)
    mask = np.asarray(inputs["mask"], dtype=np.float32)
    w1 = np.asarray(inputs["w1"], dtype=np.float32)
    b1 = np.asarray(inputs["b1"], dtype=np.float32)
    w2 = np.asarray(inputs["w2"], dtype=np.float32)
    b2 = np.asarray(inputs["b2"], dtype=np.float32)
    dw1 = np.asarray(inputs["dw1"], dtype=np.float32)
    db1 = np.asarray(inputs["db1"], dtype=np.float32)
    dw2 = np.asarray(inputs["dw2"], dtype=np.float32)
    db2 = np.asarray(inputs["db2"], dtype=np.float32)
    dw3 = np.asarray(inputs["dw3"], dtype=np.float32)
    db3 = np.asarray(inputs["db3"], dtype=np.float32)

    g, k = idx.shape
    assert g == G

    # Fold gather + grouped Dense(1) into a dense [S, GP] matrix.
    a_mat = np.zeros((S, GP), dtype=np.float32)
    gwm = (gw * mask).astype(np.float32)
    cols = np.repeat(np.arange(g, dtype=np.int64), k)
    np.add.at(a_mat, (idx.reshape(-1), cols), gwm.reshape(-1))

    bootpack = np.zeros((128, BOOT_COLS), dtype=np.float32)
    bootpack[:, IDENT_BASE : IDENT_BASE + 128] = np.eye(128, dtype=np.float32)
    gb_pad = np.zeros(GP, np.float32)
    gb_pad[:g] = gb
    for i in range(4):
        bootpack[:, BIAS_BASE + i] = gb_pad[i * 128 : (i + 1) * 128]
    bootpack[:, BIAS_BASE + 4] = b1
    bootpack[:LAT, BIAS_BASE + 5] = b2
    bootpack[:, BIAS_BASE + 6] = db1
    bootpack[:, BIAS_BASE + 7] = db2[:128]
    bootpack[:, BIAS_BASE + 8] = db2[128:]

    apack = np.zeros((128, A_COLS), dtype=BF16_NP)
    for fi in range(4):
        apack[:, fi * 512 : (fi + 1) * 512] = \
            a_mat[fi * 128 : (fi + 1) * 128, :].astype(BF16_NP)

    restpack = np.zeros((128, REST_COLS), dtype=BF16_NP)
    w1_pad = np.zeros((GP, HID), dtype=np.float32)
    w1_pad[:g] = w1
    for fi in range(4):
        restpack[:, W1_BASE + fi * 128 : W1_BASE + (fi + 1) * 128] = \
            w1_pad[fi * 128 : (fi + 1) * 128, :].astype(BF16_NP)
    restpack[:, W2_BASE : W2_BASE + LAT] = w2.astype(BF16_NP)
    restpack[:LAT, DW1_BASE : DW1_BASE + HID] = dw1.astype(BF16_NP)
    restpack[:, DW2_BASE : DW2_BASE + HID2] = dw2.astype(BF16_NP)
    for kj in range(2):
        restpack[:, DW3_BASE + kj * 512 : DW3_BASE + (kj + 1) * 512] = \
            dw3[kj * 128 : (kj + 1) * 128, :].astype(BF16_NP)
    restpack[0, ONES_BASE : ONES_BASE + 128] = 1.0
    restpack[0, DB3_BASE : DB3_BASE + S] = db3.astype(BF16_NP)

    return bootpack, apack, restpack


def kernel(**inputs) -> np.ndarray:
    global last_results

    x = np.ascontiguousarray(np.asarray(inputs["x"], dtype=np.float32))
    assert x.shape == (B, S), x.shape
    bootpack, apack, restpack = _pack_params(inputs)

    in_maps = [
        {"x": x[c * BC : (c + 1) * BC], "bootpack": bootpack,
         "apack": apack, "restpack": restpack}
        for c in range(NCORES)
    ]

    nc = _get_program()
    trace = os.environ.get("KERNEL_TRACE", "0") == "1"
    res = run_bass_kernel_spmd(nc, in_maps, list(range(NCORES)), trace=trace)
    last_results = res
    out = np.concatenate([r["out"] for r in res.results], axis=0)
    return out.astype(np.float32)


if __name__ == "__main__":
    rng = np.random.RandomState(0)
    demo = {
        "x": rng.rand(B, S).astype(np.float32),
        "gw": rng.randn(G, 30).astype(np.float32),
        "gb": rng.randn(G).astype(np.float32) * 0.1,
        "idx": rng.randint(0, S, (G, 30)).astype(np.int32),
        "mask": (rng.rand(G, 30) > 0.5).astype(np.float32),
        "w1": rng.randn(G, HID).astype(np.float32) * 0.04,
        "b1": rng.randn(HID).astype(np.float32) * 0.1,
        "w2": rng.randn(HID, LAT).astype(np.float32) * 0.09,
        "b2": rng.randn(LAT).astype(np.float32) * 0.1,
        "dw1": rng.randn(LAT, HID).astype(np.float32) * 0.18,
        "db1": rng.randn(HID).astype(np.float32) * 0.1,
        "dw2": rng.randn(HID, HID2).astype(np.float32) * 0.09,
        "db2": rng.randn(HID2).astype(np.float32) * 0.1,
        "dw3": rng.randn(HID2, S).astype(np.float32) * 0.06,
        "db3": rng.randn(S).astype(np.float32) * 0.1,
    }
    out = kernel(**demo)
    print("out", out.shape, out.dtype, float(out.mean()))
